# revision 9
# baseline (speedup 1.0000x reference)
"""MinusAttention kernel for Trainium2 (8 NeuronCores, Bass/Tile).

Math: score[i,j] = (w.q_i - w.k_j + b) / sqrt(E) with causal mask.
Within a softmax row i, the w.q_i and b terms are constant across j and
cancel, so

    weights[i,j] = g_j / sum_{j'<=i} g_j',   g_j = exp(-w.k_j / sqrt(E))
    out[i,:]     = (sum_{j<=i} g_j V[j,:]) / (sum_{j<=i} g_j)

i.e. a causal cumulative weighted average of V -- O(S*E) per (b,h)
instead of O(L*S*E) -- and the output does not depend on queries at all.

Device kernel (per core, 4 of the 32 (b,h) pairs), entirely in
transposed layout [E partitions, S free] so the causal cumsum is a
single hardware scan along the free dim:

  - kt[e,s] = -w[e]/sqrt(E) * K^T[e,s] (host-prescaled), padded with a
    zero row 64
  - skb = partition_all_reduce_add(kt)    # GPSIMD: sk broadcast to all rows
  - gb  = exp(skb)                        # ACT, [65, S]
  - W   = gb * vT_aug                     # DVE; vT_aug row 64 is ones,
                                          # so W row 64 = g
  - C   = scan_add(W)                     # DVE prefix sum along s:
                                          # rows 0..63 = cum(g*V), row 64 = cum(g)
  - r    = reciprocal(C[64])              # DVE row op
  - rb   = partition_broadcast(r)         # GPSIMD
  - outT = C[0:64] * rb                   # DVE multiply

Host reassembles the [pair, E, S] outputs into [B, L, H, E].
"""

import numpy as np

B, L, S, H, E = 4, 2048, 2048, 8, 64
NCORES = 8
PAIRS = (B * H) // NCORES  # (b,h) pairs per core
SCALE = np.float32(1.0 / np.sqrt(np.float32(E)))

# test.py can flip this to capture an NTFF profile; the harness never does.
TRACE = False
LAST_RESULTS = None

_compiled = None


def _build():
    from concourse import bacc
    import concourse.bass_isa as bass_isa
    import concourse.mybir as mybir
    import concourse.tile as tile

    f32 = mybir.dt.float32
    nc = bacc.Bacc("TRN2", target_bir_lowering=False, debug=False)

    # [pair, e, s]; kt row 64 is zeros, vt row 64 is ones
    kt = nc.dram_tensor("kt", [PAIRS, E + 1, S], f32, kind="ExternalInput")
    vt = nc.dram_tensor("vt", [PAIRS, E + 1, S], f32, kind="ExternalInput")
    out = nc.dram_tensor("out", [PAIRS, E, S], f32, kind="ExternalOutput")

    with tile.TileContext(nc) as tc:
        with (
            tc.tile_pool(name="ktp", bufs=2) as ktp,
            tc.tile_pool(name="vtp", bufs=2) as vtp,
            tc.tile_pool(name="gbp", bufs=2) as gbp,
            tc.tile_pool(name="wp", bufs=2) as wp,
            tc.tile_pool(name="cp", bufs=2) as cp,
            tc.tile_pool(name="cbp", bufs=2) as cbp,
            tc.tile_pool(name="op", bufs=2) as op,
        ):
            kts, vts = [], []
            for p in range(PAIRS):
                t = ktp.tile([E + 1, S], f32, tag="kt")
                nc.sync.dma_start(out=t[:], in_=kt[p])
                kts.append(t)
                t = vtp.tile([E + 1, S], f32, tag="vt")
                nc.sync.dma_start(out=t[:], in_=vt[p])
                vts.append(t)

            gbs = []
            for p in range(PAIRS):
                gb = gbp.tile([E + 1, S], f32, tag="gb")
                # in-place: all-reduce rows -> sk everywhere, then exp
                nc.gpsimd.partition_all_reduce(
                    gb[:], kts[p][:], channels=E + 1, reduce_op=bass_isa.ReduceOp.add
                )
                nc.scalar.activation(gb[:], gb[:], mybir.ActivationFunctionType.Exp)
                gbs.append(gb)

            ws = []
            for p in range(PAIRS):
                w = wp.tile([E + 1, S], f32, tag="w")
                nc.vector.tensor_tensor(
                    out=w[:], in0=gbs[p][:], in1=vts[p][:], op=mybir.AluOpType.mult
                )
                ws.append(w)

            cs = []
            for p in range(PAIRS):
                c = cp.tile([E + 1, S], f32, tag="c")
                nc.vector.tensor_tensor_scan(
                    c[:], ws[p][:], ws[p][:], 0.0,
                    mybir.AluOpType.add, mybir.AluOpType.bypass,
                )
                cs.append(c)

            rs = []
            for p in range(PAIRS):
                r = cbp.tile([1, S], f32, tag="r")
                nc.vector.reciprocal(r[:], cs[p][E : E + 1, :])
                rs.append(r)

            cbs = []
            for p in range(PAIRS):
                cb = cbp.tile([E, S], f32, tag="cb")
                nc.gpsimd.partition_broadcast(cb[:], rs[p][:])
                cbs.append(cb)

            for p in range(PAIRS):
                o = op.tile([E, S], f32, tag="o")
                nc.vector.tensor_tensor(
                    out=o[:], in0=cs[p][0:E, :], in1=cbs[p][:], op=mybir.AluOpType.mult
                )
                nc.sync.dma_start(out=out[p], in_=o[:])

    nc.compile()
    return nc


def _get_compiled():
    global _compiled
    if _compiled is None:
        _compiled = _build()
    return _compiled


def prep_inputs(keys: np.ndarray, values: np.ndarray, w_score: np.ndarray):
    """Host-side reshard: returns in_maps (list of 8 dicts)."""
    keys = np.asarray(keys, dtype=np.float32)
    values = np.asarray(values, dtype=np.float32)
    w = np.asarray(w_score, dtype=np.float32)

    # [B,S,H,E] -> [B,H,E,S] = [32, 64, 2048], prescale keys by -w/sqrt(E)
    ktw = keys.transpose(0, 2, 3, 1).reshape(B * H, E, S)
    ktw = ktw * (-SCALE * w)[None, :, None]
    ktw = np.concatenate([ktw, np.zeros((B * H, 1, S), np.float32)], axis=1)

    vtw = values.transpose(0, 2, 3, 1).reshape(B * H, E, S)
    vtw = np.concatenate([vtw, np.ones((B * H, 1, S), np.float32)], axis=1)

    in_maps = []
    for c in range(NCORES):
        sl = slice(PAIRS * c, PAIRS * (c + 1))
        in_maps.append(
            {
                "kt": np.ascontiguousarray(ktw[sl]),
                "vt": np.ascontiguousarray(vtw[sl]),
            }
        )
    return in_maps


def assemble_output(results) -> np.ndarray:
    # results[c]["out"]: [PAIRS, E, S]
    arr = np.stack([np.asarray(r["out"]) for r in results])  # [8, PAIRS, E, S]
    arr = arr.reshape(B, H, E, S).transpose(0, 3, 1, 2)  # [B, L, H, E]
    return np.ascontiguousarray(arr)


def kernel(queries=None, keys=None, values=None, w_score=None, b_score=None, attn_mask=None, **_):
    global LAST_RESULTS
    from concourse.bass_utils import run_bass_kernel_spmd

    nc = _get_compiled()
    in_maps = prep_inputs(keys, values, w_score)
    res = run_bass_kernel_spmd(nc, in_maps, core_ids=list(range(NCORES)), trace=TRACE)
    LAST_RESULTS = res
    return assemble_output(res.results)


# revision 15
# speedup vs baseline: 2.5247x; 2.5247x over previous
"""MinusAttention kernel for Trainium2 (8 NeuronCores, Bass/Tile).

Math: score[i,j] = (w.q_i - w.k_j + b) / sqrt(E) with causal mask.
Within a softmax row i, the w.q_i and b terms are constant across j and
cancel, so

    weights[i,j] = g_j / sum_{j'<=i} g_j',   g_j = exp(-w.k_j / sqrt(E))
    out[i,:]     = (sum_{j<=i} g_j V[j,:]) / (sum_{j<=i} g_j)

i.e. a causal cumulative weighted average of V -- O(S*E) per (b,h)
instead of O(L*S*E) -- and the output does not depend on queries at all.

Device kernel per core (4 of the 32 (b,h) pairs), natural layout
[s%128 partitions, (s//128, e) free], per pair:

  - sk[p,k]   = reduce_add_e(ktw[p,k,e])      # DVE; ktw host-prescaled by -w/sqrt(E)
  - g         = exp(sk)                       # ACT  [128,16]
  - wg        = vg * g                        # DVE TT, g broadcast along free;
                                              # vg col 64 is ones -> wg col 64 = g
  - per chunk c (4 blocks): PSUM_c = TriUT @ wg_c   (within-block prefix sums)
  - cw32      = copy(PSUM rows 96:128)        # ACT (PSUM reads must be 32-aligned)
  - bsT[k]    = cw32 row 31 of each block     # tiny SBUF->SBUF DMA
  - rhs_m     = maskT * bsT_bcast             # DVE; maskT[k',k]=1 iff k'<k
  - PSUM_c   += ones16 @ rhs_m_c              # adds carry_k = sum_{k'<k} bs_k'
  - cw        = copy(PSUM)                    # ACT -> SBUF
  - r         = 1/cw[:, :, 64]                # DVE [128,16]
  - out       = cw[:, :, 0:64] * r_bcast      # DVE TT
"""

import numpy as np

B, L, S, H, E = 4, 2048, 2048, 8, 64
NCORES = 8
PAIRS = (B * H) // NCORES  # (b,h) pairs per core
NBLK = S // 128  # 16
CHUNK = 4  # blocks per PSUM tile: 4*65 = 260 fp32 < 512 (one bank)
NCHUNK = NBLK // CHUNK  # 4
SCALE = np.float32(1.0 / np.sqrt(np.float32(E)))

TRACE = False
LAST_RESULTS = None

_compiled = None


def _build():
    from concourse import bacc
    import concourse.mybir as mybir
    import concourse.tile as tile
    from concourse.masks import make_upper_triangular

    f32 = mybir.dt.float32
    nc = bacc.Bacc("TRN2", target_bir_lowering=False, debug=False)

    ktw = nc.dram_tensor("ktw", [PAIRS, 128, NBLK, E], f32, kind="ExternalInput")
    vg = nc.dram_tensor("vg", [PAIRS, 128, NBLK, E + 1], f32, kind="ExternalInput")
    out = nc.dram_tensor("out", [PAIRS, 128, NBLK, E], f32, kind="ExternalOutput")

    with tile.TileContext(nc) as tc:
        with (
            tc.tile_pool(name="const", bufs=1) as cpool,
            tc.tile_pool(name="ktp", bufs=2) as ktp,
            tc.tile_pool(name="vgp", bufs=2) as vgp,
            tc.tile_pool(name="gp", bufs=2) as gp,
            tc.tile_pool(name="wgp", bufs=2) as wgp,
            tc.tile_pool(name="bsp", bufs=2) as bsp,
            tc.tile_pool(name="rmp", bufs=2) as rmp,
            tc.tile_pool(name="cwp", bufs=2) as cwp,
            tc.tile_pool(name="rp", bufs=2) as rp,
            tc.tile_pool(name="outp", bufs=2) as outp,
            tc.tile_pool(name="ps", bufs=8, space="PSUM") as psp,
        ):
            tri = cpool.tile([128, 128], f32)
            make_upper_triangular(nc, tri[:], val=1.0, diag=True)
            ones16 = cpool.tile([16, 128], f32)
            nc.gpsimd.memset(ones16[:], 1.0)
            # maskT[k', k, n] = 1 iff k' < k (strictly below target block)
            maskT = cpool.tile([16, NBLK, E + 1], f32)
            nc.gpsimd.memset(maskT[:], 1.0)
            nc.gpsimd.affine_select(
                out=maskT[:],
                in_=maskT[:],
                compare_op=mybir.AluOpType.is_gt,
                fill=0.0,
                base=0,
                # expr = -k' + k > 0  <=>  k' < k
                pattern=[[1, NBLK], [0, E + 1]],
                channel_multiplier=-1,
            )

            for p in range(PAIRS):
                kt = ktp.tile([128, NBLK, E], f32, tag="kt")
                nc.sync.dma_start(out=kt[:], in_=ktw[p])
                vgt = vgp.tile([128, NBLK, E + 1], f32, tag="vg")
                nc.sync.dma_start(out=vgt[:], in_=vg[p])

                g = gp.tile([128, NBLK], f32, tag="g")
                nc.vector.tensor_reduce(
                    g[:], kt[:], mybir.AxisListType.X, mybir.AluOpType.add
                )
                nc.scalar.activation(g[:], g[:], mybir.ActivationFunctionType.Exp)

                wg = wgp.tile([128, NBLK, E + 1], f32, tag="wg")
                gb = g[:].to_broadcast([128, NBLK, E + 1])
                nc.vector.tensor_tensor(out=wg[:], in0=vgt[:], in1=gb, op=mybir.AluOpType.mult)

                pss = []
                bsT = bsp.tile([NBLK, 1, E + 1], f32, tag="bs")
                for c in range(NCHUNK):
                    ps = psp.tile([128, CHUNK, E + 1], f32, tag="ps")
                    nc.tensor.matmul(
                        ps[:], lhsT=tri[:],
                        rhs=wg[:, c * CHUNK : (c + 1) * CHUNK, :],
                        start=True, stop=False, skip_group_check=True,
                    )
                    # block sums live in row 127 of each block's prefix
                    # sums; PSUM reads need 32-aligned bases, so copy rows
                    # 96:128 to SBUF and DMA row 31 out of that
                    c32 = cwp.tile([32, CHUNK, E + 1], f32, tag="cw32")
                    nc.scalar.copy(c32[:], ps[96:128, :, :])
                    # partition-scatter: src [1,(4,65)] elements land on 4
                    # partitions of bsT in iteration order
                    nc.sync.dma_start(
                        out=bsT[c * CHUNK : (c + 1) * CHUNK, :, :],
                        in_=c32[31:32, :, :],
                    )
                    pss.append(ps)

                rhs_m = rmp.tile([16, NBLK, E + 1], f32, tag="rm")
                nc.vector.tensor_tensor(
                    out=rhs_m[:], in0=maskT[:],
                    in1=bsT[:].broadcast_to([NBLK, NBLK, E + 1]),
                    op=mybir.AluOpType.mult,
                )

                cw = cwp.tile([128, NBLK, E + 1], f32, tag="cw")
                for c in range(NCHUNK):
                    nc.tensor.matmul(
                        pss[c][:], lhsT=ones16[:],
                        rhs=rhs_m[:, c * CHUNK : (c + 1) * CHUNK, :],
                        start=False, stop=True, skip_group_check=True,
                    )
                    nc.scalar.copy(cw[:, c * CHUNK : (c + 1) * CHUNK, :], pss[c][:])

                r = rp.tile([128, NBLK], f32, tag="r")
                nc.vector.reciprocal(r[:], cw[:, :, E : E + 1].rearrange("p k o -> p (k o)"))
                ot = outp.tile([128, NBLK, E], f32, tag="out")
                rb = r[:].to_broadcast([128, NBLK, E])
                nc.vector.tensor_tensor(
                    out=ot[:], in0=cw[:, :, 0:E], in1=rb, op=mybir.AluOpType.mult
                )
                nc.sync.dma_start(out=out[p], in_=ot[:])

    nc.compile()
    return nc


def _get_compiled():
    global _compiled
    if _compiled is None:
        _compiled = _build()
    return _compiled


def prep_inputs(keys: np.ndarray, values: np.ndarray, w_score: np.ndarray):
    """Host-side reshard: returns in_maps (list of 8 dicts)."""
    keys = np.asarray(keys, dtype=np.float32)
    values = np.asarray(values, dtype=np.float32)
    w = np.asarray(w_score, dtype=np.float32)

    # [B,S,H,E] -> [B,H,S,E] -> [B*H, NBLK, 128, E] -> [B*H, 128, NBLK, E]
    kt = keys.transpose(0, 2, 1, 3).reshape(B * H, NBLK, 128, E)
    kt = (kt * (-SCALE * w)).transpose(0, 2, 1, 3)

    v = values.transpose(0, 2, 1, 3).reshape(B * H, NBLK, 128, E)
    vg = np.concatenate([v, np.ones((B * H, NBLK, 128, 1), np.float32)], axis=-1)
    vg = vg.transpose(0, 2, 1, 3)  # [B*H, 128, NBLK, E+1]

    in_maps = []
    for c in range(NCORES):
        sl = slice(PAIRS * c, PAIRS * (c + 1))
        in_maps.append(
            {
                "ktw": np.ascontiguousarray(kt[sl]),
                "vg": np.ascontiguousarray(vg[sl]),
            }
        )
    return in_maps


def assemble_output(results) -> np.ndarray:
    # results[c]["out"]: [PAIRS, 128, NBLK, E]; s = 128*k + partition
    arr = np.stack([np.asarray(r["out"]) for r in results])  # [8, PAIRS, 128, NBLK, E]
    arr = arr.reshape(B * H, 128, NBLK, E).transpose(0, 2, 1, 3)  # [B*H, NBLK, 128, E]
    arr = arr.reshape(B, H, L, E).transpose(0, 2, 1, 3)  # [B, L, H, E]
    return np.ascontiguousarray(arr)


def kernel(queries=None, keys=None, values=None, w_score=None, b_score=None, attn_mask=None, **_):
    global LAST_RESULTS
    from concourse.bass_utils import run_bass_kernel_spmd

    nc = _get_compiled()
    in_maps = prep_inputs(keys, values, w_score)
    res = run_bass_kernel_spmd(nc, in_maps, core_ids=list(range(NCORES)), trace=TRACE)
    LAST_RESULTS = res
    return assemble_output(res.results)


# revision 18
# speedup vs baseline: 2.6664x; 1.0561x over previous
"""MinusAttention kernel for Trainium2 (8 NeuronCores, Bass/Tile).

Math: score[i,j] = (w.q_i - w.k_j + b) / sqrt(E) with causal mask.
Within a softmax row i, the w.q_i and b terms are constant across j and
cancel, so

    weights[i,j] = g_j / sum_{j'<=i} g_j',   g_j = exp(-w.k_j / sqrt(E))
    out[i,:]     = (sum_{j<=i} g_j V[j,:]) / (sum_{j<=i} g_j)

i.e. a causal cumulative weighted average of V -- O(S*E) per (b,h)
instead of O(L*S*E) -- and the output does not depend on queries at all.

Device kernel per core (4 of the 32 (b,h) pairs), natural layout
[s%128 partitions, (s//128, e) free], per pair:

  - sk[p,k]   = reduce_add_e(ktw[p,k,e])      # DVE; ktw host-prescaled by -w/sqrt(E)
  - g         = exp(sk)                       # ACT  [128,16]
  - wg        = vg * g                        # DVE TT, g broadcast along free;
                                              # vg col 64 is ones -> wg col 64 = g
  - per chunk c (4 blocks): PSUM_c = TriUT @ wg_c   (within-block prefix sums)
  - cw32      = copy(PSUM rows 96:128)        # ACT (PSUM reads must be 32-aligned)
  - bsT[k]    = cw32 row 31 of each block     # tiny SBUF->SBUF DMA
  - rhs_m     = maskT * bsT_bcast             # DVE; maskT[k',k]=1 iff k'<k
  - PSUM_c   += ones16 @ rhs_m_c              # adds carry_k = sum_{k'<k} bs_k'
  - cw        = copy(PSUM)                    # ACT -> SBUF
  - r         = 1/cw[:, :, 64]                # DVE [128,16]
  - out       = cw[:, :, 0:64] * r_bcast      # DVE TT
"""

import numpy as np

B, L, S, H, E = 4, 2048, 2048, 8, 64
NCORES = 8
PAIRS = (B * H) // NCORES  # (b,h) pairs per core
NBLK = S // 128  # 16
CHUNK = 4  # blocks per PSUM tile: 4*65 = 260 fp32 < 512 (one bank)
NCHUNK = NBLK // CHUNK  # 4
SCALE = np.float32(1.0 / np.sqrt(np.float32(E)))

TRACE = False
LAST_RESULTS = None

_compiled = None


def _build():
    from concourse import bacc
    import concourse.mybir as mybir
    import concourse.tile as tile
    from concourse.masks import make_upper_triangular

    f32 = mybir.dt.float32
    f32r = mybir.dt.float32r
    nc = bacc.Bacc("TRN2", target_bir_lowering=False, debug=False)

    ktw = nc.dram_tensor("ktw", [PAIRS, 128, NBLK, E], f32, kind="ExternalInput")
    vg = nc.dram_tensor("vg", [PAIRS, 128, NBLK, E + 1], f32, kind="ExternalInput")
    out = nc.dram_tensor("out", [PAIRS, 128, NBLK, E], f32, kind="ExternalOutput")

    with tile.TileContext(nc) as tc:
        with (
            tc.tile_pool(name="const", bufs=1) as cpool,
            tc.tile_pool(name="ktp", bufs=2) as ktp,
            tc.tile_pool(name="vgp", bufs=2) as vgp,
            tc.tile_pool(name="gp", bufs=2) as gp,
            tc.tile_pool(name="wgp", bufs=2) as wgp,
            tc.tile_pool(name="bsp", bufs=2) as bsp,
            tc.tile_pool(name="rmp", bufs=2) as rmp,
            tc.tile_pool(name="cwp", bufs=2) as cwp,
            tc.tile_pool(name="rp", bufs=2) as rp,
            tc.tile_pool(name="outp", bufs=2) as outp,
            tc.tile_pool(name="ps", bufs=8, space="PSUM") as psp,
        ):
            tri_f = cpool.tile([128, 128], f32)
            make_upper_triangular(nc, tri_f[:], val=1.0, diag=True)
            tri = cpool.tile([128, 128], f32r)
            nc.scalar.copy(tri[:], tri_f[:])
            ones16_f = cpool.tile([16, 128], f32)
            nc.gpsimd.memset(ones16_f[:], 1.0)
            ones16 = cpool.tile([16, 128], f32r)
            nc.scalar.copy(ones16[:], ones16_f[:])
            # maskT[k', k, n] = 1 iff k' < k (strictly below target block)
            maskT = cpool.tile([16, NBLK, E + 1], f32)
            nc.gpsimd.memset(maskT[:], 1.0)
            nc.gpsimd.affine_select(
                out=maskT[:],
                in_=maskT[:],
                compare_op=mybir.AluOpType.is_gt,
                fill=0.0,
                base=0,
                # expr = -k' + k > 0  <=>  k' < k
                pattern=[[1, NBLK], [0, E + 1]],
                channel_multiplier=-1,
            )

            for p in range(PAIRS):
                kt = ktp.tile([128, NBLK, E], f32, tag="kt")
                nc.sync.dma_start(out=kt[:], in_=ktw[p])
                vgt = vgp.tile([128, NBLK, E + 1], f32, tag="vg")
                nc.sync.dma_start(out=vgt[:], in_=vg[p])

                g = gp.tile([128, NBLK], f32, tag="g")
                nc.vector.tensor_reduce(
                    g[:], kt[:], mybir.AxisListType.X, mybir.AluOpType.add
                )
                nc.scalar.activation(g[:], g[:], mybir.ActivationFunctionType.Exp)

                wg = wgp.tile([128, NBLK, E + 1], f32r, tag="wg")
                gb = g[:].to_broadcast([128, NBLK, E + 1])
                nc.vector.tensor_tensor(out=wg[:], in0=vgt[:], in1=gb, op=mybir.AluOpType.mult)

                pss = []
                bsT = bsp.tile([NBLK, 1, E + 1], f32, tag="bs")
                for c in range(NCHUNK):
                    ps = psp.tile([128, CHUNK, E + 1], f32, tag="ps")
                    nc.tensor.matmul(
                        ps[:], lhsT=tri[:],
                        rhs=wg[:, c * CHUNK : (c + 1) * CHUNK, :],
                        start=True, stop=False, skip_group_check=True,
                    )
                    # block sums live in row 127 of each block's prefix
                    # sums; PSUM reads need 32-aligned bases, so copy rows
                    # 96:128 to SBUF and DMA row 31 out of that
                    c32 = cwp.tile([32, CHUNK, E + 1], f32, tag="cw32")
                    nc.scalar.copy(c32[:], ps[96:128, :, :])
                    # partition-scatter: src [1,(4,65)] elements land on 4
                    # partitions of bsT in iteration order
                    nc.sync.dma_start(
                        out=bsT[c * CHUNK : (c + 1) * CHUNK, :, :],
                        in_=c32[31:32, :, :],
                    )
                    pss.append(ps)

                rhs_m = rmp.tile([16, NBLK, E + 1], f32r, tag="rm")
                nc.vector.tensor_tensor(
                    out=rhs_m[:], in0=maskT[:],
                    in1=bsT[:].broadcast_to([NBLK, NBLK, E + 1]),
                    op=mybir.AluOpType.mult,
                )

                cw = cwp.tile([128, NBLK, E + 1], f32, tag="cw")
                for c in range(NCHUNK):
                    nc.tensor.matmul(
                        pss[c][:], lhsT=ones16[:],
                        rhs=rhs_m[:, c * CHUNK : (c + 1) * CHUNK, :],
                        start=False, stop=True, skip_group_check=True,
                    )
                    nc.scalar.copy(cw[:, c * CHUNK : (c + 1) * CHUNK, :], pss[c][:])

                r = rp.tile([128, NBLK], f32, tag="r")
                nc.vector.reciprocal(r[:], cw[:, :, E : E + 1].rearrange("p k o -> p (k o)"))
                ot = outp.tile([128, NBLK, E], f32, tag="out")
                rb = r[:].to_broadcast([128, NBLK, E])
                nc.vector.tensor_tensor(
                    out=ot[:], in0=cw[:, :, 0:E], in1=rb, op=mybir.AluOpType.mult
                )
                nc.sync.dma_start(out=out[p], in_=ot[:])

    nc.compile()
    return nc


def _get_compiled():
    global _compiled
    if _compiled is None:
        _compiled = _build()
    return _compiled


def prep_inputs(keys: np.ndarray, values: np.ndarray, w_score: np.ndarray):
    """Host-side reshard: returns in_maps (list of 8 dicts)."""
    keys = np.asarray(keys, dtype=np.float32)
    values = np.asarray(values, dtype=np.float32)
    w = np.asarray(w_score, dtype=np.float32)

    # [B,S,H,E] -> [B,H,S,E] -> [B*H, NBLK, 128, E] -> [B*H, 128, NBLK, E]
    kt = keys.transpose(0, 2, 1, 3).reshape(B * H, NBLK, 128, E)
    kt = (kt * (-SCALE * w)).transpose(0, 2, 1, 3)

    v = values.transpose(0, 2, 1, 3).reshape(B * H, NBLK, 128, E)
    vg = np.concatenate([v, np.ones((B * H, NBLK, 128, 1), np.float32)], axis=-1)
    vg = vg.transpose(0, 2, 1, 3)  # [B*H, 128, NBLK, E+1]

    in_maps = []
    for c in range(NCORES):
        sl = slice(PAIRS * c, PAIRS * (c + 1))
        in_maps.append(
            {
                "ktw": np.ascontiguousarray(kt[sl]),
                "vg": np.ascontiguousarray(vg[sl]),
            }
        )
    return in_maps


def assemble_output(results) -> np.ndarray:
    # results[c]["out"]: [PAIRS, 128, NBLK, E]; s = 128*k + partition
    arr = np.stack([np.asarray(r["out"]) for r in results])  # [8, PAIRS, 128, NBLK, E]
    arr = arr.reshape(B * H, 128, NBLK, E).transpose(0, 2, 1, 3)  # [B*H, NBLK, 128, E]
    arr = arr.reshape(B, H, L, E).transpose(0, 2, 1, 3)  # [B, L, H, E]
    return np.ascontiguousarray(arr)


def kernel(queries=None, keys=None, values=None, w_score=None, b_score=None, attn_mask=None, **_):
    global LAST_RESULTS
    from concourse.bass_utils import run_bass_kernel_spmd

    nc = _get_compiled()
    in_maps = prep_inputs(keys, values, w_score)
    res = run_bass_kernel_spmd(nc, in_maps, core_ids=list(range(NCORES)), trace=TRACE)
    LAST_RESULTS = res
    return assemble_output(res.results)
